# revision 5
# baseline (speedup 1.0000x reference)
"""Chamfer distance loss kernel for Trainium2 (8 NeuronCores).

Problem: B=4, N=8192, C=3. loss = mean_i min_j d[i,j] + mean_j min_i d[i,j]
over per-batch 8192x8192 squared-distance matrices.

Strategy:
  - 8 cores: core c handles batch c//2, target-row half c%2 (4096 rows x 8192 cols).
  - Host pre-augments inputs so the PE produces the distance matrix directly:
      d = lhsT.T @ rhs,  d[i,j] = |t_i|^2 + |x_j|^2 - 2 t_i.x_j
    Values are hi/lo bf16-split (v = hi + lo) and the product expanded to
    hi*hi + lo*hi + hi*lo (lo*lo dropped, ~2^-18 relative) -> K=13 bf16
    matmul with near-fp32 accuracy at 1 cycle/row.
  - Per-tile [128,2048] post-processing (static schedule, period 32):
    30/32 tiles: ScalarE drains PSUM f32 -> SBUF bf16 (1892ns); VectorE
    row-min via tensor_scalar 4x + accum_out (594ns).  2/32 tiles: VectorE
    tensor_scalar 1x drains PSUM with the row-min fused (2258ns), giving
    ScalarE headroom.  All col-min accumulation (tensor_tensor min, 1127ns)
    on VectorE.  neuronxcc forbids GPSIMD PSUM access and TensorTensor /
    accum ops, so the Pool engine cannot take any of this work.
  - dist1 row-mins exact to bf16(d); col-min partial [128, 8192] folded on host.
"""

import numpy as np

N_CORES = 8
P = 128
MM_N = 512  # matmul free width (one PSUM bank of f32)
JGW = 2048  # j-group width (4 banks)
FLT_BIG = 3.0e38
K_AUG = 13  # hi/lo-split augmented contraction depth

# Static per-32-tile schedule: slots where VectorE drains PSUM itself with
# the row-min fused (tensor_scalar 1x + accum_out), relieving ScalarE.
# neuronxcc forbids GPSIMD from touching PSUM and from TensorTensor/accum
# ops entirely, so the Pool engine cannot participate.
BETA_SET = frozenset({13, 29})

_NC_CACHE = {}


def _build(rows, ncols):
    import concourse.bacc as bacc
    import concourse.mybir as mybir
    from concourse.tile import TileContext
    from contextlib import ExitStack

    f32 = mybir.dt.float32
    bf16 = mybir.dt.bfloat16
    MIN = mybir.AluOpType.min
    nb = rows // P
    ng = ncols // JGW
    ks = JGW // MM_N

    nc = bacc.Bacc(None, target_bir_lowering=False)

    aug_t_d = nc.dram_tensor("aug_t", [K_AUG, rows], bf16, kind="ExternalInput")
    aug_x_d = nc.dram_tensor("aug_x", [K_AUG, ncols], bf16, kind="ExternalInput")
    dist1_d = nc.dram_tensor("dist1", [P, nb], f32, kind="ExternalOutput")
    colacc_d = nc.dram_tensor("colacc", [P, ncols], bf16, kind="ExternalOutput")

    with TileContext(nc) as tc, ExitStack() as ctx:
        singles = ctx.enter_context(tc.tile_pool(name="singles", bufs=1))
        psum_pool = ctx.enter_context(
            tc.tile_pool(name="psum_pool", bufs=2, space="PSUM")
        )
        dpool = ctx.enter_context(tc.tile_pool(name="dpool", bufs=6))
        spool = ctx.enter_context(tc.tile_pool(name="spool", bufs=2))

        aug_t_sb = singles.tile([K_AUG, rows], bf16)
        aug_x_sb = singles.tile([K_AUG, ncols], bf16)
        nc.sync.dma_start(out=aug_t_sb, in_=aug_t_d[:, :])
        nc.sync.dma_start(out=aug_x_sb, in_=aug_x_d[:, :])
        rowmin_buf = singles.tile([P, nb * ng], f32)
        dist1_sb = singles.tile([P, nb], f32)
        colV = [singles.tile([P, JGW], bf16, name=f"colV_{g}") for g in range(ng)]
        for g in range(ng):
            nc.vector.memset(colV[g], FLT_BIG)

        tidx = 0
        for b in range(nb):
            for g in range(ng):
                psum = psum_pool.tile([P, JGW], f32, tag="ps", name=f"ps_{b}_{g}")
                for k in range(ks):
                    nc.tensor.matmul(
                        psum[:, k * MM_N : (k + 1) * MM_N],
                        lhsT=aug_t_sb[:, b * P : (b + 1) * P],
                        rhs=aug_x_sb[
                            :, g * JGW + k * MM_N : g * JGW + (k + 1) * MM_N
                        ],
                        start=True,
                        stop=True,
                    )
                dtile = dpool.tile([P, JGW], bf16, tag="dt", name=f"dt_{b}_{g}")
                col = b * ng + g
                slot = tidx % 32
                if slot in BETA_SET:
                    nc.vector.tensor_scalar(
                        dtile,
                        psum,
                        FLT_BIG,
                        None,
                        op0=MIN,
                        op1=MIN,
                        accum_out=rowmin_buf[:, col : col + 1],
                    )
                else:
                    nc.scalar.activation(
                        dtile, psum, mybir.ActivationFunctionType.Copy
                    )
                    scr = spool.tile([P, JGW], bf16, tag="scr", name=f"sc_{b}_{g}")
                    nc.vector.tensor_scalar(
                        scr,
                        dtile,
                        FLT_BIG,
                        None,
                        op0=MIN,
                        op1=MIN,
                        accum_out=rowmin_buf[:, col : col + 1],
                    )
                nc.vector.tensor_tensor(colV[g], colV[g], dtile, MIN)
                tidx += 1

        for g in range(ng):
            nc.sync.dma_start(
                out=colacc_d[:, g * JGW : (g + 1) * JGW], in_=colV[g]
            )
        nc.vector.tensor_reduce(
            dist1_sb,
            rowmin_buf.rearrange("p (b g) -> p b g", g=ng),
            axis=mybir.AxisListType.X,
            op=MIN,
        )
        nc.sync.dma_start(out=dist1_d[:, :], in_=dist1_sb)

    return nc


def _get_nc(rows, ncols):
    key = (rows, ncols)
    if key not in _NC_CACHE:
        nc = _build(rows, ncols)
        nc.compile()
        _NC_CACHE[key] = nc
    return _NC_CACHE[key]


def _split_hi_lo(v):
    import ml_dtypes

    hi = v.astype(ml_dtypes.bfloat16)
    lo = (v - hi.astype(np.float32)).astype(ml_dtypes.bfloat16)
    return hi, lo


def _make_aug(t, x):
    """t: [R,3] f32, x: [N,3] f32 -> (aug_t [13,R] bf16, aug_x [13,N] bf16).

    d = sum_k aug_t[k].T * aug_x[k]:
      k0-2 : hi_t  *  hi_w      (w = -2x)
      k3-5 : lo_t  *  hi_w
      k6-8 : hi_t  *  lo_w
      k9   : nth   *  1         (nt = |t|^2 = nth + ntl)
      k10  : ntl   *  1
      k11  : 1     *  nxh       (nx = |x|^2 = nxh + nxl)
      k12  : 1     *  nxl
    """
    import ml_dtypes

    bf = ml_dtypes.bfloat16
    R = t.shape[0]
    N = x.shape[0]
    w = -2.0 * x
    ht, lt = _split_hi_lo(t.T)  # [3, R]
    hw, lw = _split_hi_lo(w.T)  # [3, N]
    nt = (t.astype(np.float64) ** 2).sum(1).astype(np.float32)
    nx = (x.astype(np.float64) ** 2).sum(1).astype(np.float32)
    nth, ntl = _split_hi_lo(nt)
    nxh, nxl = _split_hi_lo(nx)

    aug_t = np.empty((K_AUG, R), bf)
    aug_t[0:3] = ht
    aug_t[3:6] = lt
    aug_t[6:9] = ht
    aug_t[9] = nth
    aug_t[10] = ntl
    aug_t[11] = bf(1.0)
    aug_t[12] = bf(1.0)

    aug_x = np.empty((K_AUG, N), bf)
    aug_x[0:3] = hw
    aug_x[3:6] = hw
    aug_x[6:9] = lw
    aug_x[9] = bf(1.0)
    aug_x[10] = bf(1.0)
    aug_x[11] = nxh
    aug_x[12] = nxl
    return aug_t, aug_x


def _make_in_maps(tp, xh):
    B, N, _ = tp.shape
    half = N // 2
    aug_xs = [_make_aug(tp[b, :1], xh[b])[1] for b in range(B)]
    in_maps = []
    for c in range(N_CORES):
        bidx, h = divmod(c, 2)
        t = tp[bidx, h * half : (h + 1) * half]  # [half, 3]
        aug_t, _ = _make_aug(t, xh[bidx, :1])
        in_maps.append({"aug_t": aug_t, "aug_x": aug_xs[bidx]})
    return in_maps


def _combine(results, B, N):
    d1_sum = 0.0
    d2_sum = 0.0
    for bidx in range(B):
        ccs = []
        for h in range(2):
            r = results[2 * bidx + h]
            d1 = np.asarray(r["dist1"]).astype(np.float64)
            d1_sum += float(d1.sum())
            ccs.append(np.asarray(r["colacc"]).astype(np.float32))
        m = np.minimum(ccs[0], ccs[1]).min(axis=0)
        d2_sum += float(m.astype(np.float64).sum())
    return np.float32(d1_sum / (B * N) + d2_sum / (B * N))


def _run(inputs, trace=False):
    tp = np.ascontiguousarray(np.asarray(inputs["target_pos"], np.float32))
    xh = np.ascontiguousarray(np.asarray(inputs["x_hat"], np.float32))
    B, N, _ = tp.shape
    half = N // 2
    in_maps = _make_in_maps(tp, xh)
    nc = _get_nc(half, N)
    from concourse.bass_utils import run_bass_kernel_spmd

    res = run_bass_kernel_spmd(
        nc, in_maps, list(range(N_CORES)), trace=trace
    )
    loss = _combine(res.results, B, N)
    return loss, res


def kernel(**inputs) -> np.ndarray:
    loss, _ = _run(inputs)
    return loss


# revision 8
# speedup vs baseline: 1.0004x; 1.0004x over previous
"""Chamfer distance loss kernel for Trainium2 (8 NeuronCores).

Problem: B=4, N=8192, C=3. loss = mean_i min_j d[i,j] + mean_j min_i d[i,j]
over per-batch 8192x8192 squared-distance matrices.

Strategy:
  - 8 cores: core c handles batch c//2, target-row half c%2 (4096 rows x 8192 cols).
  - Host pre-augments inputs so the PE produces the distance matrix directly:
      d = lhsT.T @ rhs,  d[i,j] = |t_i|^2 + |x_j|^2 - 2 t_i.x_j
    Values are hi/lo bf16-split (v = hi + lo) and the product expanded to
    hi*hi + lo*hi + hi*lo (lo*lo dropped, ~2^-18 relative) -> K=13 bf16
    matmul with near-fp32 accuracy at 1 cycle/row.
  - Per-tile [128,2048] post-processing (static schedule, period 32):
    30/32 tiles: ScalarE drains PSUM f32 -> SBUF bf16 (1892ns); VectorE
    row-min via tensor_scalar 4x + accum_out (594ns).  2/32 tiles: VectorE
    tensor_scalar 1x drains PSUM with the row-min fused (2258ns), giving
    ScalarE headroom.  All col-min accumulation (tensor_tensor min, 1127ns)
    on VectorE.  neuronxcc forbids GPSIMD PSUM access and TensorTensor /
    accum ops, so the Pool engine cannot take any of this work.
  - dist1 row-mins exact to bf16(d); col-min partial [128, 8192] folded on host.
"""

import numpy as np

N_CORES = 8
P = 128
MM_N = 512  # matmul free width (one PSUM bank of f32)
JGW = 2048  # j-group width (4 banks)
FLT_BIG = 3.0e38
K_AUG = 13  # hi/lo-split augmented contraction depth

# Static per-32-tile schedule: slots where VectorE drains PSUM itself with
# the row-min fused (tensor_scalar 1x + accum_out), relieving ScalarE.
# neuronxcc forbids GPSIMD from touching PSUM and from TensorTensor/accum
# ops entirely, so the Pool engine cannot participate.
BETA_SET = frozenset({13, 29})

_NC_CACHE = {}


def _build(rows, ncols):
    import concourse.bacc as bacc
    import concourse.mybir as mybir
    from concourse.tile import TileContext
    from contextlib import ExitStack

    f32 = mybir.dt.float32
    bf16 = mybir.dt.bfloat16
    MIN = mybir.AluOpType.min
    nb = rows // P
    ng = ncols // JGW
    ks = JGW // MM_N

    nc = bacc.Bacc(None, target_bir_lowering=False)

    aug_t_d = nc.dram_tensor("aug_t", [K_AUG, rows], bf16, kind="ExternalInput")
    aug_x_d = nc.dram_tensor("aug_x", [K_AUG, ncols], bf16, kind="ExternalInput")
    dist1_d = nc.dram_tensor("dist1", [P, nb], f32, kind="ExternalOutput")
    colacc_d = nc.dram_tensor("colacc", [P, ncols], bf16, kind="ExternalOutput")

    with TileContext(nc) as tc, ExitStack() as ctx:
        singles = ctx.enter_context(tc.tile_pool(name="singles", bufs=1))
        psum_pool = ctx.enter_context(
            tc.tile_pool(name="psum_pool", bufs=2, space="PSUM")
        )
        dpool = ctx.enter_context(tc.tile_pool(name="dpool", bufs=6))
        spool = ctx.enter_context(tc.tile_pool(name="spool", bufs=2))

        aug_t_sb = singles.tile([K_AUG, rows], bf16)
        aug_x_sb = singles.tile([K_AUG, ncols], bf16)
        nc.sync.dma_start(out=aug_t_sb, in_=aug_t_d[:, :])
        nc.sync.dma_start(out=aug_x_sb, in_=aug_x_d[:, :])
        rowmin_buf = singles.tile([P, nb * ng], f32)
        dist1_sb = singles.tile([P, nb], f32)
        colV = [singles.tile([P, JGW], bf16, name=f"colV_{g}") for g in range(ng)]
        for g in range(ng):
            nc.gpsimd.memset(colV[g], FLT_BIG)

        tidx = 0
        for b in range(nb):
            for g in range(ng):
                psum = psum_pool.tile([P, JGW], f32, tag="ps", name=f"ps_{b}_{g}")
                for k in range(ks):
                    nc.tensor.matmul(
                        psum[:, k * MM_N : (k + 1) * MM_N],
                        lhsT=aug_t_sb[:, b * P : (b + 1) * P],
                        rhs=aug_x_sb[
                            :, g * JGW + k * MM_N : g * JGW + (k + 1) * MM_N
                        ],
                        start=True,
                        stop=True,
                    )
                dtile = dpool.tile([P, JGW], bf16, tag="dt", name=f"dt_{b}_{g}")
                col = b * ng + g
                slot = tidx % 32
                if slot in BETA_SET:
                    nc.vector.tensor_scalar(
                        dtile,
                        psum,
                        FLT_BIG,
                        None,
                        op0=MIN,
                        op1=MIN,
                        accum_out=rowmin_buf[:, col : col + 1],
                    )
                else:
                    nc.scalar.activation(
                        dtile, psum, mybir.ActivationFunctionType.Copy
                    )
                    scr = spool.tile([P, JGW], bf16, tag="scr", name=f"sc_{b}_{g}")
                    nc.vector.tensor_scalar(
                        scr,
                        dtile,
                        FLT_BIG,
                        None,
                        op0=MIN,
                        op1=MIN,
                        accum_out=rowmin_buf[:, col : col + 1],
                    )
                nc.vector.tensor_tensor(colV[g], colV[g], dtile, MIN)
                tidx += 1

        for g in range(ng):
            nc.sync.dma_start(
                out=colacc_d[:, g * JGW : (g + 1) * JGW], in_=colV[g]
            )
        nc.vector.tensor_reduce(
            dist1_sb,
            rowmin_buf.rearrange("p (b g) -> p b g", g=ng),
            axis=mybir.AxisListType.X,
            op=MIN,
        )
        nc.sync.dma_start(out=dist1_d[:, :], in_=dist1_sb)

    return nc


def _get_nc(rows, ncols):
    key = (rows, ncols)
    if key not in _NC_CACHE:
        nc = _build(rows, ncols)
        nc.compile()
        _NC_CACHE[key] = nc
    return _NC_CACHE[key]


def _split_hi_lo(v):
    import ml_dtypes

    hi = v.astype(ml_dtypes.bfloat16)
    lo = (v - hi.astype(np.float32)).astype(ml_dtypes.bfloat16)
    return hi, lo


def _make_aug(t, x):
    """t: [R,3] f32, x: [N,3] f32 -> (aug_t [13,R] bf16, aug_x [13,N] bf16).

    d = sum_k aug_t[k].T * aug_x[k]:
      k0-2 : hi_t  *  hi_w      (w = -2x)
      k3-5 : lo_t  *  hi_w
      k6-8 : hi_t  *  lo_w
      k9   : nth   *  1         (nt = |t|^2 = nth + ntl)
      k10  : ntl   *  1
      k11  : 1     *  nxh       (nx = |x|^2 = nxh + nxl)
      k12  : 1     *  nxl
    """
    import ml_dtypes

    bf = ml_dtypes.bfloat16
    R = t.shape[0]
    N = x.shape[0]
    w = -2.0 * x
    ht, lt = _split_hi_lo(t.T)  # [3, R]
    hw, lw = _split_hi_lo(w.T)  # [3, N]
    nt = (t.astype(np.float64) ** 2).sum(1).astype(np.float32)
    nx = (x.astype(np.float64) ** 2).sum(1).astype(np.float32)
    nth, ntl = _split_hi_lo(nt)
    nxh, nxl = _split_hi_lo(nx)

    aug_t = np.empty((K_AUG, R), bf)
    aug_t[0:3] = ht
    aug_t[3:6] = lt
    aug_t[6:9] = ht
    aug_t[9] = nth
    aug_t[10] = ntl
    aug_t[11] = bf(1.0)
    aug_t[12] = bf(1.0)

    aug_x = np.empty((K_AUG, N), bf)
    aug_x[0:3] = hw
    aug_x[3:6] = hw
    aug_x[6:9] = lw
    aug_x[9] = bf(1.0)
    aug_x[10] = bf(1.0)
    aug_x[11] = nxh
    aug_x[12] = nxl
    return aug_t, aug_x


def _make_in_maps(tp, xh):
    B, N, _ = tp.shape
    half = N // 2
    aug_xs = [_make_aug(tp[b, :1], xh[b])[1] for b in range(B)]
    in_maps = []
    for c in range(N_CORES):
        bidx, h = divmod(c, 2)
        t = tp[bidx, h * half : (h + 1) * half]  # [half, 3]
        aug_t, _ = _make_aug(t, xh[bidx, :1])
        in_maps.append({"aug_t": aug_t, "aug_x": aug_xs[bidx]})
    return in_maps


def _combine(results, B, N):
    d1_sum = 0.0
    d2_sum = 0.0
    for bidx in range(B):
        ccs = []
        for h in range(2):
            r = results[2 * bidx + h]
            d1 = np.asarray(r["dist1"]).astype(np.float64)
            d1_sum += float(d1.sum())
            ccs.append(np.asarray(r["colacc"]).astype(np.float32))
        m = np.minimum(ccs[0], ccs[1]).min(axis=0)
        d2_sum += float(m.astype(np.float64).sum())
    return np.float32(d1_sum / (B * N) + d2_sum / (B * N))


def _run(inputs, trace=False):
    tp = np.ascontiguousarray(np.asarray(inputs["target_pos"], np.float32))
    xh = np.ascontiguousarray(np.asarray(inputs["x_hat"], np.float32))
    B, N, _ = tp.shape
    half = N // 2
    in_maps = _make_in_maps(tp, xh)
    nc = _get_nc(half, N)
    from concourse.bass_utils import run_bass_kernel_spmd

    res = run_bass_kernel_spmd(
        nc, in_maps, list(range(N_CORES)), trace=trace
    )
    loss = _combine(res.results, B, N)
    return loss, res


def kernel(**inputs) -> np.ndarray:
    loss, _ = _run(inputs)
    return loss


# revision 9
# speedup vs baseline: 1.0056x; 1.0052x over previous
"""Chamfer distance loss kernel for Trainium2 (8 NeuronCores).

Problem: B=4, N=8192, C=3. loss = mean_i min_j d[i,j] + mean_j min_i d[i,j]
over per-batch 8192x8192 squared-distance matrices.

Strategy:
  - 8 cores: core c handles batch c//2, target-row half c%2 (4096 rows x 8192 cols).
  - Host pre-augments inputs so the PE produces the distance matrix directly:
      d = lhsT.T @ rhs,  d[i,j] = |t_i|^2 + |x_j|^2 - 2 t_i.x_j
    Values are hi/lo bf16-split (v = hi + lo) and the product expanded to
    hi*hi + lo*hi + hi*lo (lo*lo dropped, ~2^-18 relative) -> K=13 bf16
    matmul with near-fp32 accuracy at 1 cycle/row.
  - Per-tile [128,2048] post-processing (static schedule, period 32):
    30/32 tiles: ScalarE drains PSUM f32 -> SBUF bf16 (1892ns); VectorE
    row-min via tensor_scalar 4x + accum_out (594ns).  2/32 tiles: VectorE
    tensor_scalar 1x drains PSUM with the row-min fused (2258ns), giving
    ScalarE headroom.  All col-min accumulation (tensor_tensor min, 1127ns)
    on VectorE.  neuronxcc forbids GPSIMD PSUM access and TensorTensor /
    accum ops, so the Pool engine cannot take any of this work.
  - dist1 row-mins exact to bf16(d); col-min partial [128, 8192] folded on host.
"""

import numpy as np

N_CORES = 8
P = 128
MM_N = 512  # matmul free width (one PSUM bank of f32)
JGW = 2048  # j-group width (4 banks)
FLT_BIG = 3.0e38
K_AUG = 13  # hi/lo-split augmented contraction depth

# Static per-32-tile schedule: slots where VectorE drains PSUM itself with
# the row-min fused (tensor_scalar 1x + accum_out), relieving ScalarE.
# neuronxcc forbids GPSIMD from touching PSUM and from TensorTensor/accum
# ops entirely, so the Pool engine cannot participate.
BETA_SET = frozenset({5, 21})

_NC_CACHE = {}


def _build(rows, ncols):
    import concourse.bacc as bacc
    import concourse.mybir as mybir
    from concourse.tile import TileContext
    from contextlib import ExitStack

    f32 = mybir.dt.float32
    bf16 = mybir.dt.bfloat16
    MIN = mybir.AluOpType.min
    nb = rows // P
    ng = ncols // JGW
    ks = JGW // MM_N

    nc = bacc.Bacc(None, target_bir_lowering=False)

    aug_t_d = nc.dram_tensor("aug_t", [K_AUG, rows], bf16, kind="ExternalInput")
    aug_x_d = nc.dram_tensor("aug_x", [K_AUG, ncols], bf16, kind="ExternalInput")
    dist1_d = nc.dram_tensor("dist1", [P, nb], f32, kind="ExternalOutput")
    colacc_d = nc.dram_tensor("colacc", [P, ncols], bf16, kind="ExternalOutput")

    with TileContext(nc) as tc, ExitStack() as ctx:
        singles = ctx.enter_context(tc.tile_pool(name="singles", bufs=1))
        psum_pool = ctx.enter_context(
            tc.tile_pool(name="psum_pool", bufs=2, space="PSUM")
        )
        dpool = ctx.enter_context(tc.tile_pool(name="dpool", bufs=6))
        spool = ctx.enter_context(tc.tile_pool(name="spool", bufs=2))

        aug_t_sb = singles.tile([K_AUG, rows], bf16)
        aug_x_sb = singles.tile([K_AUG, ncols], bf16)
        nc.sync.dma_start(out=aug_t_sb, in_=aug_t_d[:, :])
        nc.sync.dma_start(out=aug_x_sb, in_=aug_x_d[:, :])
        rowmin_buf = singles.tile([P, nb * ng], f32)
        dist1_sb = singles.tile([P, nb], f32)
        colV = [singles.tile([P, JGW], bf16, name=f"colV_{g}") for g in range(ng)]
        for g in range(ng):
            nc.gpsimd.memset(colV[g], FLT_BIG)

        tidx = 0
        for b in range(nb):
            for g in range(ng):
                psum = psum_pool.tile([P, JGW], f32, tag="ps", name=f"ps_{b}_{g}")
                for k in range(ks):
                    nc.tensor.matmul(
                        psum[:, k * MM_N : (k + 1) * MM_N],
                        lhsT=aug_t_sb[:, b * P : (b + 1) * P],
                        rhs=aug_x_sb[
                            :, g * JGW + k * MM_N : g * JGW + (k + 1) * MM_N
                        ],
                        start=True,
                        stop=True,
                    )
                dtile = dpool.tile([P, JGW], bf16, tag="dt", name=f"dt_{b}_{g}")
                col = b * ng + g
                slot = tidx % 32
                if slot in BETA_SET:
                    nc.vector.tensor_scalar(
                        dtile,
                        psum,
                        FLT_BIG,
                        None,
                        op0=MIN,
                        op1=MIN,
                        accum_out=rowmin_buf[:, col : col + 1],
                    )
                else:
                    nc.scalar.activation(
                        dtile, psum, mybir.ActivationFunctionType.Copy
                    )
                    scr = spool.tile([P, JGW], bf16, tag="scr", name=f"sc_{b}_{g}")
                    nc.vector.tensor_scalar(
                        scr,
                        dtile,
                        FLT_BIG,
                        None,
                        op0=MIN,
                        op1=MIN,
                        accum_out=rowmin_buf[:, col : col + 1],
                    )
                nc.vector.tensor_tensor(colV[g], colV[g], dtile, MIN)
                tidx += 1

        for g in range(ng):
            nc.sync.dma_start(
                out=colacc_d[:, g * JGW : (g + 1) * JGW], in_=colV[g]
            )
        nc.vector.tensor_reduce(
            dist1_sb,
            rowmin_buf.rearrange("p (b g) -> p b g", g=ng),
            axis=mybir.AxisListType.X,
            op=MIN,
        )
        nc.sync.dma_start(out=dist1_d[:, :], in_=dist1_sb)

    return nc


def _get_nc(rows, ncols):
    key = (rows, ncols)
    if key not in _NC_CACHE:
        nc = _build(rows, ncols)
        nc.compile()
        _NC_CACHE[key] = nc
    return _NC_CACHE[key]


def _split_hi_lo(v):
    import ml_dtypes

    hi = v.astype(ml_dtypes.bfloat16)
    lo = (v - hi.astype(np.float32)).astype(ml_dtypes.bfloat16)
    return hi, lo


def _make_aug(t, x):
    """t: [R,3] f32, x: [N,3] f32 -> (aug_t [13,R] bf16, aug_x [13,N] bf16).

    d = sum_k aug_t[k].T * aug_x[k]:
      k0-2 : hi_t  *  hi_w      (w = -2x)
      k3-5 : lo_t  *  hi_w
      k6-8 : hi_t  *  lo_w
      k9   : nth   *  1         (nt = |t|^2 = nth + ntl)
      k10  : ntl   *  1
      k11  : 1     *  nxh       (nx = |x|^2 = nxh + nxl)
      k12  : 1     *  nxl
    """
    import ml_dtypes

    bf = ml_dtypes.bfloat16
    R = t.shape[0]
    N = x.shape[0]
    w = -2.0 * x
    ht, lt = _split_hi_lo(t.T)  # [3, R]
    hw, lw = _split_hi_lo(w.T)  # [3, N]
    nt = (t.astype(np.float64) ** 2).sum(1).astype(np.float32)
    nx = (x.astype(np.float64) ** 2).sum(1).astype(np.float32)
    nth, ntl = _split_hi_lo(nt)
    nxh, nxl = _split_hi_lo(nx)

    aug_t = np.empty((K_AUG, R), bf)
    aug_t[0:3] = ht
    aug_t[3:6] = lt
    aug_t[6:9] = ht
    aug_t[9] = nth
    aug_t[10] = ntl
    aug_t[11] = bf(1.0)
    aug_t[12] = bf(1.0)

    aug_x = np.empty((K_AUG, N), bf)
    aug_x[0:3] = hw
    aug_x[3:6] = hw
    aug_x[6:9] = lw
    aug_x[9] = bf(1.0)
    aug_x[10] = bf(1.0)
    aug_x[11] = nxh
    aug_x[12] = nxl
    return aug_t, aug_x


def _make_in_maps(tp, xh):
    B, N, _ = tp.shape
    half = N // 2
    aug_xs = [_make_aug(tp[b, :1], xh[b])[1] for b in range(B)]
    in_maps = []
    for c in range(N_CORES):
        bidx, h = divmod(c, 2)
        t = tp[bidx, h * half : (h + 1) * half]  # [half, 3]
        aug_t, _ = _make_aug(t, xh[bidx, :1])
        in_maps.append({"aug_t": aug_t, "aug_x": aug_xs[bidx]})
    return in_maps


def _combine(results, B, N):
    d1_sum = 0.0
    d2_sum = 0.0
    for bidx in range(B):
        ccs = []
        for h in range(2):
            r = results[2 * bidx + h]
            d1 = np.asarray(r["dist1"]).astype(np.float64)
            d1_sum += float(d1.sum())
            ccs.append(np.asarray(r["colacc"]).astype(np.float32))
        m = np.minimum(ccs[0], ccs[1]).min(axis=0)
        d2_sum += float(m.astype(np.float64).sum())
    return np.float32(d1_sum / (B * N) + d2_sum / (B * N))


def _run(inputs, trace=False):
    tp = np.ascontiguousarray(np.asarray(inputs["target_pos"], np.float32))
    xh = np.ascontiguousarray(np.asarray(inputs["x_hat"], np.float32))
    B, N, _ = tp.shape
    half = N // 2
    in_maps = _make_in_maps(tp, xh)
    nc = _get_nc(half, N)
    from concourse.bass_utils import run_bass_kernel_spmd

    res = run_bass_kernel_spmd(
        nc, in_maps, list(range(N_CORES)), trace=trace
    )
    loss = _combine(res.results, B, N)
    return loss, res


def kernel(**inputs) -> np.ndarray:
    loss, _ = _run(inputs)
    return loss
